# revision 1
# baseline (speedup 1.0000x reference)
"""DCRNN forward kernel for 8 Trainium2 NeuronCores (Bass/Tile).

Sharding: data-parallel over batch (B=8 -> 1 element/core, zero communication).
Each core runs the full 24-cell encoder+decoder recurrence with all supports,
weights, and state SBUF-resident.

Math decomposition (validated in numpy, rel err 6e-6 vs the jax reference):
  supports: Sa = rw(adj).T, Sb = rw(adj.T).T.  Device holds SAT := rw(adj),
  SBT := rw(adj.T) and QA := 2*SAT@SAT - I (= (2 Sa^2 - I).T), QB likewise,
  so the K=2 Chebyshev second hop is one matmul (no chained diffusion).
  State is feature-major ("T-form"): hT (128 feats, 1024 nodes).  Per gconv:
    z = transpose(hT)                          8 PE-transpose shots
    hdiff_mT = z.T @ MAT, MAT in [SAT,QA,SBT,QB]   (lhsT=z chunks, rhs=MAT rows,
               free dim 512 -> full-rate float32r)
    valT = sum_m Wh_m.T @ hdiff_mT + Wx.T @ XdT + bias   (lhsT=packed weights,
           rhs=diffusion outputs; no activation transposes)
  x-channel: encoder x-diffusions batched over all 12 frames up front; decoder
  x_t = h_{t-1}@pW + pb folds onto the gate h-diffusions:
  v_m = pW.T @ hdiff_mT + pb*colsums(MAT).
"""
import os
import sys

sys.path.insert(0, "/opt/trn_rl_repo")

import numpy as np
from contextlib import ExitStack

import concourse.tile as tile
from concourse import bacc, mybir

N, U, SEQ, HOR, NM = 1024, 128, 12, 12, 5
P = 128
KC = N // P          # 8 contraction chunks over nodes
NB = N // 512        # 2 free-dim chunks of 512 over nodes
F32 = mybir.dt.float32
F32R = mybir.dt.float32r
AFT = mybir.ActivationFunctionType

LAST_EXEC_NS = None
LAST_RESULTS = None
_COMPILED = None


def _chunk_pack(a):
    """(1024, C) -> (128, 8*C); matrix rows [128k,128k+128) land at cols [kC,(k+1)C)."""
    C = a.shape[1]
    return np.ascontiguousarray(
        a.reshape(KC, P, C).transpose(1, 0, 2).reshape(P, KC * C)
    )


def _host_prep(inputs, adj, enc_Wg, enc_bg, enc_Wc, enc_bc,
               dec_Wg, dec_bg, dec_Wc, dec_bc, proj_W, proj_b):
    f32 = np.float32
    adj = np.asarray(adj, f32)

    def rw(a):
        d = a.sum(1)
        dinv = np.where(d > 0, 1.0 / d, 0.0).astype(f32)
        return (dinv[:, None] * a).astype(f32)

    SAT = rw(adj)
    SBT = rw(np.ascontiguousarray(adj.T))
    eye = np.eye(N, dtype=f32)
    QA = (2.0 * (SAT @ SAT) - eye).astype(f32)
    QB = (2.0 * (SBT @ SBT) - eye).astype(f32)
    pb = float(np.asarray(proj_b).reshape(-1)[0])
    beta97 = np.zeros((P, N), f32)
    for m, MAT in enumerate([SAT, QA, SBT, QB]):
        beta97[32 * m] = pb * MAT.sum(0)

    def packWh(W):
        W = np.asarray(W, f32)
        out = W.shape[1]
        return np.ascontiguousarray(W[NM:].reshape(U, NM * out))

    def packWx(W):
        return np.ascontiguousarray(np.asarray(W, f32)[:NM])

    def stack_bias(b, n_chunks):
        b = np.asarray(b, f32)
        return np.ascontiguousarray(
            np.stack([b[i * P:(i + 1) * P] for i in range(n_chunks)], axis=1))

    smallc = np.zeros((P, 1024), f32)
    smallc[0:NM, 0:256] = packWx(enc_Wg)
    smallc[0:NM, 256:384] = packWx(enc_Wc)
    smallc[0:NM, 384:640] = packWx(dec_Wg)
    smallc[0:NM, 640:768] = packWx(dec_Wc)
    smallc[:, 768:896] = np.eye(P, dtype=f32)
    smallc[:, 896:898] = stack_bias(dec_bg, 2)
    smallc[:, 898:899] = stack_bias(dec_bc, 1)
    smallc[:, 899:901] = stack_bias(enc_bg, 2)
    smallc[:, 901:902] = stack_bias(enc_bc, 1)
    smallc[:, 902:903] = np.asarray(proj_W, f32)
    smallc[:, 903:904] = pb
    shared = {
        "SAT": _chunk_pack(SAT), "QA": _chunk_pack(QA),
        "SBT": _chunk_pack(SBT), "QB": _chunk_pack(QB),
        "eWg": packWh(enc_Wg), "eWc": packWh(enc_Wc),
        "dWg": packWh(dec_Wg), "dWc": packWh(dec_Wc),
        "beta97": beta97,
        "smallc": smallc,
    }
    per_core = [{"Xin": _chunk_pack(np.asarray(inputs[b], f32))} for b in range(8)]
    return shared, per_core


_INPUT_SPECS = {
    "SAT": (P, KC * N), "QA": (P, KC * N), "SBT": (P, KC * N), "QB": (P, KC * N),
    "eWg": (U, NM * 2 * U), "eWc": (U, NM * U),
    "dWg": (U, NM * 2 * U), "dWc": (U, NM * U),
    "beta97": (P, N), "smallc": (P, 1024), "Xin": (P, KC * SEQ),
}


def _build():
    nc = bacc.Bacc("TRN2", target_bir_lowering=False, debug=False, num_devices=8)
    io = {name: nc.dram_tensor(name, list(shape),
                               F32 if name == "beta97" else F32R,
                               kind="ExternalInput").ap()
          for name, shape in _INPUT_SPECS.items()}
    out_dram = nc.dram_tensor("out", [P, KC * HOR], F32, kind="ExternalOutput").ap()
    with tile.TileContext(nc) as tc:
        _emit(tc, io, out_dram)
    nc.compile()
    return nc


def _emit(tc, io, out_dram):
    nc = tc.nc
    ctx = ExitStack()

    cpool = ctx.enter_context(tc.tile_pool(name="const", bufs=1))
    work = ctx.enter_context(tc.tile_pool(name="work", bufs=1))
    st_pool = ctx.enter_context(tc.tile_pool(name="state", bufs=2))
    gd_pool = ctx.enter_context(tc.tile_pool(name="gdiff", bufs=4))
    cd_pool = gd_pool   # candidate diffs reuse the gate-diff slots (disjoint lifetimes)
    xd_pool = ctx.enter_context(tc.tile_pool(name="xd", bufs=1))
    psA = ctx.enter_context(tc.tile_pool(name="psA", bufs=2, space="PSUM"))
    psW = ctx.enter_context(tc.tile_pool(name="psW", bufs=2, space="PSUM"))

    def const(name):
        t = cpool.tile(list(_INPUT_SPECS[name]),
                       F32 if name == "beta97" else F32R, tag=name)
        nc.sync.dma_start(t[:], io[name][:])
        return t

    MATS = [const("SAT"), const("QA"), const("SBT"), const("QB")]
    eWg, eWc, dWg, dWc = const("eWg"), const("eWc"), const("dWg"), const("dWc")
    beta97 = const("beta97")
    sc = const("smallc")
    Xin = sc[:, 904:1000]
    nc.sync.dma_start(Xin, io["Xin"][:])
    eWgx, eWcx = sc[0:NM, 0:256], sc[0:NM, 256:384]
    dWgx, dWcx = sc[0:NM, 384:640], sc[0:NM, 640:768]
    ident = sc[:, 768:896]
    dbg, dbc = sc[:, 896:898], sc[:, 898:899]
    ebg, ebc = sc[:, 899:901], sc[:, 901:902]
    pW, pb128 = sc[:, 902:903], sc[:, 903:904]
    pb1 = sc[0:1, 903:904]

    def MM(out_ap, lhsT_ap, rhs_ap, start=True, stop=True, tile_position=None):
        nc.tensor.matmul(out_ap, lhsT_ap.bitcast(F32R), rhs_ap.bitcast(F32R),
                         start=start, stop=stop, tile_position=tile_position)

    def TR(out_ap, in_ap):
        nc.tensor.transpose(out_ap.bitcast(F32R), in_ap.bitcast(F32R),
                            ident.bitcast(F32R))

    copy_eng = [lambda o, i: nc.scalar.copy(o, i),
                lambda o, i: nc.vector.tensor_copy(o, i)]

    # ------------- encoder x-channel precompute (all 12 frames) -----------
    # XT_enc (12,1024): frames as T-form rows.  X14 (128,1024): diffusion of
    # frame t by MAT m sits at partition 32m+t.
    XT_enc = work.tile([SEQ, N], F32R, tag="XT_enc")
    X14 = work.tile([P, N], F32R, tag="X14")
    for k in range(KC):
        pst = psA.tile([SEQ, P], F32, tag="psA")
        nc.tensor.transpose(pst[:, :].bitcast(F32),
                            Xin[:, k * SEQ:(k + 1) * SEQ].bitcast(F32),
                            ident.bitcast(F32))
        copy_eng[k % 2](XT_enc[:, k * P:(k + 1) * P], pst[:, :])
    for mi, MAT in enumerate(MATS):
        pse = psA.tile([SEQ, N], F32, tag="psA")
        for k in range(KC):
            for n in range(NB):
                MM(pse[:, n * 512:(n + 1) * 512],
                   Xin[:, k * SEQ:(k + 1) * SEQ],
                   MAT[:, k * N + n * 512: k * N + n * 512 + 512],
                   start=(k == 0), stop=(k == KC - 1))
        copy_eng[mi % 2](X14[32 * mi:32 * mi + SEQ, :], pse[:, :])

    out_sb = work.tile([P, KC * HOR], F32, tag="out_sb")

    # ------------- building blocks ----------------------------------------
    def transpose_to_node(srcT):
        z = work.tile([P, N], F32R, tag="z_node")
        for k in range(KC):
            pst = psA.tile([P, P], F32, tag="psA")
            TR(pst[:, :], srcT[:, k * P:(k + 1) * P])
            copy_eng[k % 2](z[:, k * P:(k + 1) * P], pst[:, :])
        return z

    def wterm(psWs, lhsT_tile, col0, kparts, rhs, start, stop):
        """psWs[Mc] += lhsT_tile[:kparts, col0+128*Mc : ...].T @ rhs (both 512-col halves)."""
        for Mc in range(len(psWs)):
            c0 = col0 + Mc * P
            for n in range(NB):
                MM(psWs[Mc][:, n * 512:(n + 1) * 512],
                   lhsT_tile[0:kparts, c0:c0 + P],
                   rhs[0:kparts, n * 512:(n + 1) * 512],
                   start=start, stop=stop)

    def gconv(srcT, Wh, Wx, out_w, bias, act, dsts, diff_pool, xd=None,
              fill_v=False):
        """One T-form graph convolution; writes act(valT + bias) into dsts.
        xd: (5,1024) x-channel stack tile; fill_v: decoder mode -- compute the
        v-rows from this gconv's diffusions and DMA them into xd[1:5] first."""
        nMc = out_w // P
        z = transpose_to_node(srcT)
        psWs = [psW.tile([P, N], F32, tag="psW", name=f"psw{Mc}")
                for Mc in range(nMc)]
        wterm(psWs, Wh, 0 * out_w, U, srcT, start=True, stop=False)   # m=0
        if xd is not None and not fill_v:
            wterm(psWs, Wx, 0, NM, xd, start=False, stop=False)
        diffs = []
        for mi, MAT in enumerate(MATS):
            psd = psA.tile([P, N], F32, tag="psA")
            for k in range(KC):
                for n in range(NB):
                    MM(psd[:, n * 512:(n + 1) * 512],
                       z[:, k * P:(k + 1) * P],
                       MAT[:, k * N + n * 512: k * N + n * 512 + 512],
                       start=(k == 0), stop=(k == KC - 1))
            d = diff_pool.tile([P, N], F32R, tag="d")
            copy_eng[mi % 2](d[:, :], psd[:, :])
            diffs.append(d)
            last = (mi == 3) and (xd is None or (xd is not None and not fill_v))
            wterm(psWs, Wh, (mi + 1) * out_w, U, d, start=False, stop=last)
        if fill_v:
            assert xd is not None
            # v-row staging reuses the z_node slot (z is idle here)
            stg = work.tile([P, N], F32R, tag="z_node", name="stg")
            for m in range(4):
                v_ps = psA.tile([1, N], F32, tag="psA", name=f"vps{m}")
                for n in range(NB):
                    MM(v_ps[0:1, n * 512:(n + 1) * 512],
                       pW, diffs[m][:, n * 512:(n + 1) * 512])
                nc.vector.tensor_add(stg[32 * m:32 * m + 1, :],
                                     v_ps[0:1, :],
                                     beta97[32 * m:32 * m + 1, :])
                nc.sync.dma_start(xd[1 + m:2 + m, :], stg[32 * m:32 * m + 1, :])
            wterm(psWs, Wx, 0, NM, xd, start=False, stop=True)
        for Mc in range(nMc):
            nc.scalar.activation(dsts[Mc][:, :], psWs[Mc][:, :], act,
                                 bias=bias[:, Mc:Mc + 1].bitcast(F32))
        return diffs

    def gconv_h0(Wx, out_w, bias, act, dsts, xd):
        """gconv for h == 0 (first encoder cell): x-terms only."""
        nMc = out_w // P
        psWs = [psW.tile([P, N], F32, tag="psW", name=f"psw{Mc}")
                for Mc in range(nMc)]
        wterm(psWs, Wx, 0, NM, xd, start=True, stop=True)
        for Mc in range(nMc):
            nc.scalar.activation(dsts[Mc][:, :], psWs[Mc][:, :], act,
                                 bias=bias[:, Mc:Mc + 1].bitcast(F32))

    rT = work.tile([P, N], F32R, tag="rT")
    uT = work.tile([P, N], F32R, tag="uT")
    cT = work.tile([P, N], F32R, tag="cT")
    rhT = work.tile([P, N], F32R, tag="rhT")

    def finish_cell(hT_old):
        """h_new = c + u*(h-c); hT_old None => h==0 => h_new = c - u*c."""
        hT_new = st_pool.tile([P, N], F32R, tag="hT")
        scr = work.tile([P, N], F32R, tag="z_node", name="scr")  # z idle here
        if hT_old is None:
            nc.vector.tensor_mul(scr[:, :], uT[:, :], cT[:, :])
            nc.vector.tensor_sub(hT_new[:, :], cT[:, :], scr[:, :])
        else:
            nc.vector.tensor_sub(scr[:, :], hT_old[:, :], cT[:, :])
            nc.vector.tensor_mul(rT[:, :], uT[:, :], scr[:, :])   # rT dead: reuse
            nc.vector.tensor_add(hT_new[:, :], cT[:, :], rT[:, :])
        return hT_new

    def assemble_enc_xd(t):
        xd = xd_pool.tile([NM, N], F32R, tag="xd")
        nc.sync.dma_start(xd[0:1, :], XT_enc[t:t + 1, :])
        for m in range(4):
            nc.sync.dma_start(xd[1 + m:2 + m, :], X14[32 * m + t:32 * m + t + 1, :])
        return xd

    # ------------- encoder -------------------------------------------------
    xd = assemble_enc_xd(0)
    gconv_h0(eWgx, 2 * U, ebg, AFT.Sigmoid, [rT, uT], xd)
    gconv_h0(eWcx, U, ebc, AFT.Tanh, [cT], xd)
    hT = finish_cell(None)

    for t in range(1, SEQ):
        xd = assemble_enc_xd(t)
        gconv(hT, eWg, eWgx, 2 * U, ebg, AFT.Sigmoid, [rT, uT], gd_pool, xd=xd)
        nc.vector.tensor_mul(rhT[:, :], rT[:, :], hT[:, :])
        gconv(rhT, eWc, eWcx, U, ebc, AFT.Tanh, [cT], cd_pool, xd=xd)
        hT = finish_cell(hT)

    # ------------- decoder -------------------------------------------------
    xd_cur = None
    for t in range(HOR):
        gconv(hT, dWg, dWgx, 2 * U, dbg, AFT.Sigmoid, [rT, uT], gd_pool,
              xd=xd_cur, fill_v=(xd_cur is not None))
        nc.vector.tensor_mul(rhT[:, :], rT[:, :], hT[:, :])
        gconv(rhT, dWc, dWcx, U, dbc, AFT.Tanh, [cT], cd_pool, xd=xd_cur)
        hT = finish_cell(hT)
        # writeback column t: out_col = h @ pW + pb
        psc = psA.tile([P, KC], F32, tag="psA")
        for k in range(KC):
            nc.tensor.matmul(psc[:, k:k + 1],
                             hT[:, k * P:(k + 1) * P].bitcast(F32),
                             pW.bitcast(F32), start=True, stop=True)
        nc.vector.tensor_scalar_add(out_sb[:, t:KC * HOR:HOR], psc[:, :],
                                    pb128.bitcast(F32))
        if t < HOR - 1:
            # out_row + pb -> row 0 of next cell's Xd stack
            xd_next = xd_pool.tile([NM, N], F32R, tag="xd")
            psr = psA.tile([1, N], F32, tag="psA")
            for n in range(NB):
                MM(psr[0:1, n * 512:(n + 1) * 512], pW,
                   hT[:, n * 512:(n + 1) * 512])
            nc.scalar.activation(xd_next[0:1, :], psr[0:1, :], AFT.Identity,
                                 bias=pb1.bitcast(F32))
            xd_cur = xd_next

    nc.sync.dma_start(out_dram[:], out_sb[:, :])
    ctx.close()


def _install_ntff_hook():
    """Provide antenv.axon_hooks (absent in this image) so bass_utils'
    trace=True path can NTFF-profile via the axon .so.  Dev-only: the
    default KERNEL_TRACE=0 path never reaches this."""
    import types
    try:
        from antenv.axon_hooks import get_axon_ntff_profile_hook  # noqa: F401
        return
    except ImportError:
        pass
    try:
        from trn_agent_boot.trn_boot import _ntff_profile_via_ctypes
        hook = _ntff_profile_via_ctypes("/opt/axon/libaxon_pjrt.so")
    except Exception:
        hook = None
    mod = types.ModuleType("antenv.axon_hooks")
    mod.get_axon_ntff_profile_hook = lambda: hook
    mod.set_axon_ntff_profile_hook = lambda h: None
    import antenv
    antenv.axon_hooks = mod
    sys.modules["antenv.axon_hooks"] = mod


def kernel(inputs, labels, adj, enc_Wg, enc_bg, enc_Wc, enc_bc,
           dec_Wg, dec_bg, dec_Wc, dec_bc, proj_W, proj_b):
    global LAST_EXEC_NS, LAST_RESULTS, _COMPILED
    from concourse.bass_utils import run_bass_kernel_spmd

    shared, per_core = _host_prep(inputs, adj, enc_Wg, enc_bg, enc_Wc, enc_bc,
                                  dec_Wg, dec_bg, dec_Wc, dec_bc, proj_W, proj_b)
    if _COMPILED is None:
        _COMPILED = _build()
    nc = _COMPILED

    in_maps = [dict(shared, **per_core[b]) for b in range(8)]
    trace = bool(int(os.environ.get("KERNEL_TRACE", "0")))
    if trace:
        _install_ntff_hook()
    res = run_bass_kernel_spmd(nc, in_maps, list(range(8)), trace=trace)
    LAST_EXEC_NS = res.exec_time_ns
    LAST_RESULTS = res
    out = np.stack([
        res.results[b]["out"].reshape(P, KC, HOR).transpose(1, 0, 2).reshape(N, HOR)
        for b in range(8)
    ])
    return out.astype(np.float32)



# revision 8
# speedup vs baseline: 1.5267x; 1.5267x over previous
"""DCRNN forward kernel for 8 Trainium2 NeuronCores (Bass/Tile), v2.

Sharding: data-parallel over batch (B=8 -> 1 element/core, zero communication).
Each core runs the full 24-cell encoder+decoder recurrence SBUF-resident.

v2 changes vs the fp32 baseline (1.60 ms):
  * all-bf16 matmul operand path (fp32 PSUM accumulation).  Validated in a
    numpy bit-model: rel err 6e-3 vs the fp32 reference (budget 2e-2).
  * decoder x-channel folded into the gate weights:  x_t = h_{t-1} pW + pb,
    so  S_m x_t = hdiff_m pW + pb colsums(S_m).  The pW part is a rank-1
    update  Wh'_m = Wh_m + pW Wx_{1+m}  (host-side fold); the colsums part
    is a static rank-4 matmul vs beta4.  The candidate's x-term rides on the
    gate's diffusions through CF_m = pW dWcx_{1+m}.  This removes all v-row
    matmuls/row-adds and the per-cell xd assembly in the decoder.
  * per-cell software pipeline tuned against the perfetto trace: sliced
    (256-col) activations / r*h / state update feeding the next gconv's
    transposes chunk-by-chunk, per-half diffusion->wterm interleave with
    PSUM->SBUF copies in the shadow of matmuls, boundary-filler matmuls
    (x-channel/beta wterms of the NEXT gconv) so the PE never idles long
    enough (>3.4us) for the HAM clock gate to re-throttle to 1.2 GHz.
  * output written as rows via the projection matmuls that the decoder
    already needs (kills the 8 free-dim-1 column-writeback matmuls).
"""
import os
import sys

sys.path.insert(0, "/opt/trn_rl_repo")

import numpy as np
import ml_dtypes
from contextlib import ExitStack

import concourse.tile as tile
from concourse import bacc, mybir

N, U, SEQ, HOR, NM = 1024, 128, 12, 12, 5
P = 128
KC = N // P          # 8 contraction chunks over nodes
NB = N // 512        # 2 free-dim halves of 512 over nodes
SL = 256             # activation / elementwise slice width
BF = mybir.dt.bfloat16
F32 = mybir.dt.float32
AFT = mybir.ActivationFunctionType
NPBF = ml_dtypes.bfloat16

LAST_EXEC_NS = None
LAST_RESULTS = None
_COMPILED = None


def _chunk_pack(a):
    """(1024, C) -> (128, 8*C); matrix rows [128k,128k+128) land at cols [kC,(k+1)C)."""
    C = a.shape[1]
    return np.ascontiguousarray(
        a.reshape(KC, P, C).transpose(1, 0, 2).reshape(P, KC * C)
    )


def _host_prep(inputs, adj, enc_Wg, enc_bg, enc_Wc, enc_bc,
               dec_Wg, dec_bg, dec_Wc, dec_bc, proj_W, proj_b):
    f32 = np.float32
    adj = np.asarray(adj, f32)

    def rw(a):
        d = a.sum(1)
        dinv = np.where(d > 0, 1.0 / d, 0.0).astype(f32)
        return (dinv[:, None] * a).astype(f32)

    SAT = rw(adj)
    SBT = rw(np.ascontiguousarray(adj.T))
    eye = np.eye(N, dtype=f32)
    QA = (2.0 * (SAT @ SAT) - eye).astype(f32)
    QB = (2.0 * (SBT @ SBT) - eye).astype(f32)
    MATS = [SAT, QA, SBT, QB]
    pb = float(np.asarray(proj_b).reshape(-1)[0])
    pWc = np.asarray(proj_W, f32).reshape(U, 1)

    def packWh(W):
        W = np.asarray(W, f32)
        out = W.shape[1]
        return np.ascontiguousarray(W[NM:].reshape(U, NM * out))

    def packWx(W):
        return np.ascontiguousarray(np.asarray(W, f32)[:NM])

    eWg_p, eWc_p = packWh(enc_Wg), packWh(enc_Wc)
    dWg_p, dWc_p = packWh(dec_Wg), packWh(dec_Wc)
    eWgx, eWcx = packWx(enc_Wg), packWx(enc_Wc)
    dWgx, dWcx = packWx(dec_Wg), packWx(dec_Wc)

    # fold x_t = h pW + pb into the decoder gate weights (blocks m=1..4)
    dWgF = dWg_p.copy().reshape(U, NM, 2 * U)
    for m in range(1, NM):
        dWgF[:, m, :] += pWc @ dWgx[m:m + 1, :]
    dWgF = np.ascontiguousarray(dWgF.reshape(U, NM * 2 * U))
    # candidate x-term rides on the gate diffusions
    CFh = np.zeros((U, 4 * U), f32)
    for m in range(1, NM):
        CFh[:, (m - 1) * U:m * U] = pWc @ dWcx[m:m + 1, :]

    # x-channel weights: [0:5, 0:384] full rows (gate 256 | cand 128);
    # [0:4, 384:768] rows 1..4 shifted to partition base 0 (beta wterm lhsT)
    exw = np.zeros((NM, 768), f32)
    exw[:, 0:256] = eWgx
    exw[:, 256:384] = eWcx
    dxw = np.zeros((NM, 768), f32)
    dxw[:, 0:256] = dWgx
    dxw[:, 256:384] = dWcx
    dxw[0:4, 384:640] = dWgx[1:]
    dxw[0:4, 640:768] = dWcx[1:]

    beta4 = np.zeros((4, N), f32)
    for m, M in enumerate(MATS):
        beta4[m] = pb * M.sum(0)

    def stack_bias(b, n_chunks):
        b = np.asarray(b, f32)
        return np.stack([b[i * P:(i + 1) * P] for i in range(n_chunks)], axis=1)

    biases = np.zeros((P, 8), f32)
    biases[:, 0:2] = stack_bias(enc_bg, 2)
    biases[:, 2:3] = stack_bias(enc_bc, 1)
    biases[:, 3:5] = stack_bias(dec_bg, 2)
    biases[:, 5:6] = stack_bias(dec_bc, 1)
    biases[:, 6] = pb

    bf = lambda x: np.ascontiguousarray(np.asarray(x, f32)).astype(NPBF)
    shared = {
        "SAT": bf(_chunk_pack(SAT)), "QA": bf(_chunk_pack(QA)),
        "SBT": bf(_chunk_pack(SBT)), "QB": bf(_chunk_pack(QB)),
        "eWg": bf(eWg_p), "eWc": bf(eWc_p),
        "dWg": bf(dWg_p), "dWc": bf(dWc_p), "dWgF": bf(dWgF),
        "CFh": bf(CFh), "exw": bf(exw), "dxw": bf(dxw),
        "beta4": bf(beta4), "identB": bf(eye[:P, :P]),
        "pWb": bf(pWc), "biases": biases.astype(f32),
    }
    per_core = [{"Xin": bf(_chunk_pack(np.asarray(inputs[b], f32)))}
                for b in range(8)]
    return shared, per_core


_SPECS = {
    "SAT": ((P, KC * N), BF), "QA": ((P, KC * N), BF),
    "SBT": ((P, KC * N), BF), "QB": ((P, KC * N), BF),
    "eWg": ((U, NM * 2 * U), BF), "eWc": ((U, NM * U), BF),
    "dWg": ((U, NM * 2 * U), BF), "dWc": ((U, NM * U), BF),
    "dWgF": ((U, NM * 2 * U), BF),
    "CFh": ((U, 4 * U), BF), "exw": ((NM, 768), BF), "dxw": ((NM, 768), BF),
    "beta4": ((4, N), BF), "identB": ((P, P), BF), "pWb": ((P, 1), BF),
    "biases": ((P, 8), F32), "Xin": ((P, KC * SEQ), BF),
}


def _build():
    nc = bacc.Bacc("TRN2", target_bir_lowering=False, debug=False, num_devices=8)
    io = {name: nc.dram_tensor(name, list(shape), dt, kind="ExternalInput").ap()
          for name, (shape, dt) in _SPECS.items()}
    out_dram = nc.dram_tensor("out", [HOR, N], F32, kind="ExternalOutput").ap()
    with tile.TileContext(nc) as tc:
        _emit(tc, io, out_dram)
    nc.compile()
    return nc


def _emit(tc, io, out_dram):
    nc = tc.nc
    ctx = ExitStack()

    cpool = ctx.enter_context(tc.tile_pool(name="const", bufs=1))
    work = ctx.enter_context(tc.tile_pool(name="work", bufs=1))
    stp = ctx.enter_context(tc.tile_pool(name="state", bufs=2))
    dpool = ctx.enter_context(tc.tile_pool(name="dpool", bufs=4))
    xdp = ctx.enter_context(tc.tile_pool(name="xdp", bufs=2))
    prp = ctx.enter_context(tc.tile_pool(name="prp", bufs=2))
    ps = ctx.enter_context(tc.tile_pool(name="ps", bufs=2, space="PSUM"))
    pw = ctx.enter_context(tc.tile_pool(name="pw", bufs=6, space="PSUM"))

    def const(name):
        shape, dt = _SPECS[name]
        t = cpool.tile(list(shape), dt, tag=name, name=name)
        nc.sync.dma_start(t[:], io[name][:])
        return t

    # light constants first, the 2MB support matrices last (x-diffusion
    # starts as soon as each lands)
    identB = const("identB")
    Xin = const("Xin")
    exw, dxw = const("exw"), const("dxw")
    biases = const("biases")
    beta4, pWb = const("beta4"), const("pWb")
    eWg, eWc = const("eWg"), const("eWc")
    dWg, dWc, dWgF = const("dWg"), const("dWc"), const("dWgF")
    CFh = const("CFh")
    MATS = [const("SAT"), const("QA"), const("SBT"), const("QB")]

    def MM(out, lhsT, rhs, start=True, stop=True):
        nc.tensor.matmul(out, lhsT, rhs, start=start, stop=stop)

    def TRq(dst_ps, j, src_cols):
        """transpose src_cols (128 wide) into quarter j of a [*,512] PSUM bank."""
        nc.tensor.matmul(dst_ps[:, j * P:(j + 1) * P], src_cols, identB[:, :],
                         is_transpose=True, skip_group_check=True)

    scop = lambda o, i: nc.scalar.copy(o, i)
    vcop = lambda o, i: nc.vector.tensor_copy(o, i)

    # ------------- persistent work tiles ----------------------------------
    XT_enc = work.tile([SEQ, N], BF, tag="XT_enc")
    X14 = work.tile([P, N], BF, tag="X14")
    rT = work.tile([P, N], BF, tag="rT")
    uT = work.tile([P, N], BF, tag="uT")
    cT = work.tile([P, N], BF, tag="cT")
    rhT = work.tile([P, N], BF, tag="rhT")
    scr = work.tile([P, N], BF, tag="scr")

    # ------------- prologue: x transposes + x diffusions ------------------
    for h in range(NB):
        ptx = ps.tile([SEQ, 512], BF, tag="psd", name=f"ptx{h}")
        for j in range(4):
            k = 4 * h + j
            TRq(ptx, j, Xin[:, k * SEQ:(k + 1) * SEQ])
        vcop(XT_enc[:, h * 512:(h + 1) * 512], ptx[0:SEQ, :])
    for m in range(4):
        for h in range(NB):
            pse = ps.tile([SEQ, 512], F32, tag="psd", name=f"pse{m}{h}")
            for k in range(KC):
                MM(pse[0:SEQ, :], Xin[:, k * SEQ:(k + 1) * SEQ],
                   MATS[m][:, k * N + h * 512: k * N + h * 512 + 512],
                   start=(k == 0), stop=(k == KC - 1))
            (vcop if (m + h) % 2 else scop)(
                X14[32 * m:32 * m + SEQ, h * 512:(h + 1) * 512], pse[0:SEQ, :])

    def assemble_xd(t):
        xd = xdp.tile([NM, N], BF, tag="xd", name=f"xd{t}")
        nc.sync.dma_start(xd[0:1, :], XT_enc[t:t + 1, :])
        for m in range(4):
            nc.sync.dma_start(xd[1 + m:2 + m, :], X14[32 * m + t:32 * m + t + 1, :])
        return xd

    # ------------- shared cell machinery ----------------------------------
    def make_z(srcT, z, extra_after_bank=None):
        """z <- transpose(srcT), 4 packed PE transposes + 1 copy per 512 bank."""
        for h in range(NB):
            pt = ps.tile([P, 512], BF, tag="psd", name=f"zps{h}")
            for j in range(4):
                TRq(pt, j, srcT[:, (4 * h + j) * P:(4 * h + j + 1) * P])
            vcop(z[:, h * 512:(h + 1) * 512], pt[:, :])
        if extra_after_bank is not None:
            extra_after_bank()

    def diff_one(z, m, h, name):
        pd = ps.tile([P, 512], F32, tag="psd", name=f"pd{name}")
        for k in range(KC):
            MM(pd[:, :], z[:, k * P:(k + 1) * P],
               MATS[m][:, k * N + h * 512: k * N + h * 512 + 512],
               start=(k == 0), stop=(k == KC - 1))
        d = dpool.tile([P, 512], BF, tag="d", name=f"d{name}")
        (vcop if h == 0 else scop)(d[:, :], pd[:, :])
        return d

    def act_slices(dst, psrow, func, bias_col):
        """dst[:, :] = func(psW + bias), emitted as 4 SL slices (scalar)."""
        for j in range(N // SL):
            n, s = j // 2, j % 2
            nc.scalar.activation(dst[:, j * SL:(j + 1) * SL],
                                 psrow[n][:, s * SL:(s + 1) * SL], func,
                                 bias=biases[:, bias_col:bias_col + 1])

    def gconv(srcT, Wh, out_w, psg, xterm_rhs, xterm_w, extra_m=None,
              pre_diff=None, first_already=False):
        """Diffusion + wterm core.  psg: [Mc][n] PSUM tiles (groups may
        already be started by head matmuls).  xterm: optional (rhs, lhsT
        col base) wterm against exw/dxw rows 0:5."""
        nMc = out_w // P
        z = work.tile([P, N], BF, tag="z", bufs=2, name="z")
        make_z(srcT, z)
        first = not first_already
        # m=0 (identity) term
        for n in range(NB):
            for Mc in range(nMc):
                MM(psg[Mc][n], Wh[0:U, Mc * P:Mc * P + P],
                   srcT[:, n * 512:(n + 1) * 512], start=first, stop=False)
        if xterm_rhs is not None:
            for n in range(NB):
                for Mc in range(nMc):
                    MM(psg[Mc][n], xterm_w[0:NM, Mc * P:Mc * P + P],
                       xterm_rhs[0:NM, n * 512:(n + 1) * 512],
                       start=False, stop=False)
        if pre_diff is not None:
            pre_diff()
        for m in range(4):
            ds = [diff_one(z, m, h, f"{m}{h}") for h in range(NB)]
            last = (m == 3)
            for n in range(NB):
                for Mc in range(nMc):
                    MM(psg[Mc][n], Wh[0:U, (m + 1) * out_w + Mc * P:
                                      (m + 1) * out_w + Mc * P + P],
                       ds[n][:, :], start=False, stop=(last and extra_m is None))
                if extra_m is not None:
                    extra_m(m, n, ds[n])
        return z

    def finish(hT_old, hT_new, zero_h):
        """hT_new = cT + uT*(hT_old - cT)   (or cT - uT*cT when h==0)."""
        for j in range(N // SL):
            sl = slice(j * SL, (j + 1) * SL)
            if zero_h:
                nc.vector.tensor_mul(scr[:, sl], uT[:, sl], cT[:, sl])
                nc.vector.tensor_sub(hT_new[:, sl], cT[:, sl], scr[:, sl])
            else:
                nc.vector.tensor_sub(scr[:, sl], hT_old[:, sl], cT[:, sl])
                nc.vector.tensor_mul(scr[:, sl], uT[:, sl], scr[:, sl])
                nc.vector.tensor_add(hT_new[:, sl], cT[:, sl], scr[:, sl])

    def rh_slices():
        for j in range(N // SL):
            sl = slice(j * SL, (j + 1) * SL)
            nc.vector.tensor_mul(rhT[:, sl], rT[:, sl], hT[:, sl])

    def alloc_psg(nMc, pfx):
        return [[pw.tile([P, 512], F32, tag="psw", name=f"{pfx}{Mc}{n}")
                 for n in range(NB)] for Mc in range(nMc)]

    # ------------- encoder -------------------------------------------------
    xd_cur = assemble_xd(0)
    xd_nxt = assemble_xd(1)

    # cell 0: h == 0, x-channel only (r unused)
    psg = alloc_psg(1, "g")
    psc = alloc_psg(1, "c")
    for n in range(NB):
        MM(psg[0][n], exw[0:NM, P:2 * P],
           xd_cur[0:NM, n * 512:(n + 1) * 512], start=True, stop=True)
        MM(psc[0][n], exw[0:NM, 256:384],
           xd_cur[0:NM, n * 512:(n + 1) * 512], start=True, stop=True)
    act_slices(uT, psg[0], AFT.Sigmoid, 1)
    act_slices(cT, psc[0], AFT.Tanh, 2)
    hT = stp.tile([P, N], BF, tag="hT", name="hT0")
    finish(None, hT, zero_h=True)

    for t in range(1, SEQ):
        xd_cur, xd_nxt = xd_nxt, (assemble_xd(t + 1) if t + 1 < SEQ else None)
        # gate: x-channel wterm first = boundary filler for the PE
        psg = alloc_psg(2, "g")
        psc = alloc_psg(1, "c")
        for n in range(NB):
            for Mc in range(2):
                MM(psg[Mc][n], exw[0:NM, Mc * P:(Mc + 1) * P],
                   xd_cur[0:NM, n * 512:(n + 1) * 512], start=True, stop=False)
        gconv(hT, eWg, 2 * U, psg, None, None, first_already=True)
        act_slices(rT, psg[0], AFT.Sigmoid, 0)
        act_slices(uT, psg[1], AFT.Sigmoid, 1)
        # candidate: its x-channel wterm fills the gate->cand boundary
        for n in range(NB):
            MM(psc[0][n], exw[0:NM, 256:384],
               xd_cur[0:NM, n * 512:(n + 1) * 512], start=True, stop=False)
        rh_slices()
        gconv(rhT, eWc, U, psc, None, None, first_already=True)
        act_slices(cT, psc[0], AFT.Tanh, 2)
        hT_new = stp.tile([P, N], BF, tag="hT", name=f"hTe{t}")
        finish(hT, hT_new, zero_h=False)
        hT = hT_new

    # ------------- decoder -------------------------------------------------
    prow = None
    for t in range(HOR):
        dec0 = (t == 0)
        Wg_t = dWg if dec0 else dWgF
        psg = alloc_psg(2, "g")
        psc = alloc_psg(1, "c")
        if not dec0:
            # static beta wterms: boundary filler, start the psum groups
            for n in range(NB):
                for Mc in range(2):
                    MM(psg[Mc][n], dxw[0:4, 384 + Mc * P:384 + (Mc + 1) * P],
                       beta4[0:4, n * 512:(n + 1) * 512], start=True, stop=False)
                MM(psc[0][n], dxw[0:4, 640:768],
                   beta4[0:4, n * 512:(n + 1) * 512], start=True, stop=False)

        # transposes + projection row (out row t-1 == x_t row)
        z = work.tile([P, N], BF, tag="z", bufs=2, name="z")
        make_z(hT, z)
        if not dec0:
            prs = []
            for n in range(NB):
                pr = ps.tile([1, 512], F32, tag="psd", name=f"pr{n}")
                MM(pr[0:1, :], pWb[:, 0:1], hT[:, n * 512:(n + 1) * 512])
                prs.append(pr)
            prow = prp.tile([1, N], BF, tag="prow", name=f"prow{t}")
            orow = prp.tile([1, N], F32, tag="orow", name=f"orow{t}")
            for n in range(NB):
                nc.scalar.activation(prow[0:1, n * 512:(n + 1) * 512],
                                     prs[n][0:1, :], AFT.Identity,
                                     bias=biases[0:1, 6:7])
                nc.vector.tensor_scalar_add(orow[0:1, n * 512:(n + 1) * 512],
                                            prs[n][0:1, :], biases[0:1, 6:7])
            nc.sync.dma_start(out_dram[t - 1:t, :], orow[0:1, :])

        # m=0 + dynamic x-row terms
        for n in range(NB):
            for Mc in range(2):
                MM(psg[Mc][n], Wg_t[0:U, Mc * P:Mc * P + P],
                   hT[:, n * 512:(n + 1) * 512], start=dec0, stop=False)
        if not dec0:
            for n in range(NB):
                for Mc in range(2):
                    MM(psg[Mc][n], dxw[0:1, Mc * P:(Mc + 1) * P],
                       prow[0:1, n * 512:(n + 1) * 512], start=False, stop=False)
                MM(psc[0][n], dxw[0:1, 256:384],
                   prow[0:1, n * 512:(n + 1) * 512], start=False, stop=False)

        def cf_extra(m, n, d):
            MM(psc[0][n], CFh[0:U, m * U:(m + 1) * U], d[:, :],
               start=False, stop=False)

        # gate diffusion loop
        nMc = 2
        for m in range(4):
            ds = [diff_one(z, m, h, f"dg{m}{h}") for h in range(NB)]
            for n in range(NB):
                for Mc in range(nMc):
                    MM(psg[Mc][n], Wg_t[0:U, (m + 1) * 2 * U + Mc * P:
                                        (m + 1) * 2 * U + Mc * P + P],
                       ds[n][:, :], start=False, stop=(m == 3))
                if not dec0:
                    cf_extra(m, n, ds[n])
        act_slices(rT, psg[0], AFT.Sigmoid, 3)
        act_slices(uT, psg[1], AFT.Sigmoid, 4)
        rh_slices()
        # candidate (always unfolded dWc; x handled via CF/beta/x-row)
        zc = work.tile([P, N], BF, tag="z", bufs=2, name="zc")
        make_z(rhT, zc)
        for n in range(NB):
            MM(psc[0][n], dWc[0:U, 0:P], rhT[:, n * 512:(n + 1) * 512],
               start=dec0, stop=False)
        for m in range(4):
            ds = [diff_one(zc, m, h, f"dc{m}{h}") for h in range(NB)]
            for n in range(NB):
                MM(psc[0][n], dWc[0:U, (m + 1) * U:(m + 1) * U + P],
                   ds[n][:, :], start=False, stop=(m == 3))
        act_slices(cT, psc[0], AFT.Tanh, 5)
        hT_new = stp.tile([P, N], BF, tag="hT", name=f"hTd{t}")
        finish(hT, hT_new, zero_h=False)
        hT = hT_new

    # epilogue: final output row (t = HOR-1)
    orow = prp.tile([1, N], F32, tag="orow", name="orowE")
    for n in range(NB):
        pr = ps.tile([1, 512], F32, tag="psd", name=f"prE{n}")
        MM(pr[0:1, :], pWb[:, 0:1], hT[:, n * 512:(n + 1) * 512])
        nc.vector.tensor_scalar_add(orow[0:1, n * 512:(n + 1) * 512],
                                    pr[0:1, :], biases[0:1, 6:7])
    nc.sync.dma_start(out_dram[HOR - 1:HOR, :], orow[0:1, :])
    ctx.close()


def _install_ntff_hook():
    """Provide antenv.axon_hooks (absent in this image) so bass_utils'
    trace=True path can NTFF-profile via the axon .so."""
    import types
    try:
        from antenv.axon_hooks import get_axon_ntff_profile_hook  # noqa: F401
        return
    except ImportError:
        pass
    try:
        from trn_agent_boot.trn_boot import _ntff_profile_via_ctypes
        hook = _ntff_profile_via_ctypes("/opt/axon/libaxon_pjrt.so")
    except Exception:
        hook = None
    mod = types.ModuleType("antenv.axon_hooks")
    mod.get_axon_ntff_profile_hook = lambda: hook
    mod.set_axon_ntff_profile_hook = lambda h: None
    import antenv
    antenv.axon_hooks = mod
    sys.modules["antenv.axon_hooks"] = mod


def kernel(inputs, labels, adj, enc_Wg, enc_bg, enc_Wc, enc_bc,
           dec_Wg, dec_bg, dec_Wc, dec_bc, proj_W, proj_b):
    global LAST_EXEC_NS, LAST_RESULTS, _COMPILED
    from concourse.bass_utils import run_bass_kernel_spmd

    shared, per_core = _host_prep(inputs, adj, enc_Wg, enc_bg, enc_Wc, enc_bc,
                                  dec_Wg, dec_bg, dec_Wc, dec_bc, proj_W, proj_b)
    if _COMPILED is None:
        _COMPILED = _build()
    nc = _COMPILED

    in_maps = [dict(shared, **per_core[b]) for b in range(8)]
    trace = bool(int(os.environ.get("KERNEL_TRACE", "0")))
    if trace:
        _install_ntff_hook()
    res = run_bass_kernel_spmd(nc, in_maps, list(range(8)), trace=trace)
    LAST_EXEC_NS = res.exec_time_ns
    LAST_RESULTS = res
    out = np.stack([
        np.asarray(res.results[b]["out"], np.float32).reshape(HOR, N).T
        for b in range(8)
    ])
    return np.ascontiguousarray(out.astype(np.float32))
